# revision 6
# baseline (speedup 1.0000x reference)
"""Locally-connected graph-conv kernel for Trainium2 (Bass/Tile), bf16.

Computes out[b,t,m] = sum_n x[b,t,n] * (S*W)[n,m] + bias[m] for
x [64, 2048, 208], W/S [208, 208], bias [208].

The ring-graph support S is a +-4 band (mod 208), so each half of the
output nodes only needs a 112-row slice of the contraction dim:
  block 0 (m 0..103):   n in {204..207} ++ {0..107}
  block 1 (m 104..207): n in {100..207} ++ {0..3}

Tolerance is 2e-2 and bf16 end-to-end measures 4.5e-3 max rel err, so
the host pre-casts x and the masked weight to bf16 (halves HBM load
traffic), the kernel stores bf16 (halves store traffic), and the host
upcasts on gather. The 16 DMA engines saturate at ~20GB/s each
(~320GB/s/core with all 8 cores streaming; reads 17.9, writes 23.1),
so the ~14MB/core of traffic bounds the kernel at ~44us of DMA window
plus ~7us fixed preamble; everything else must hide under it:
  - setup (wh/bias, 47KB) rides the Scalar HWDGE ring ahead of the
    stores; x loads all issue up-front on the Sync ring (deep pools)
    so the load queue never idles and chunk processing is interleaved
    (block0+block1 per chunk) to start stores early and keep
    reads+writes concurrently in the memory controller;
  - PSUM evictions alternate VectorE/ScalarE per group;
  - one 8-bank PSUM pool (bufs=4) keeps 4 matmul groups in flight;
  - stores are unpadded [104, T] (the 16-engine stripe splits 104
    lines 7/6 -- mildly uneven, but saves 7% of store bytes).

Data-parallel over 8 NeuronCores: each core gets 16384 rows of the
flattened x, host-pre-assembled into a bf16 [224, 16384] tensor (two
112-row halo blocks). The host transposes/upcasts at gather.
"""

import numpy as np
from contextlib import ExitStack

import concourse.bacc as bacc
import concourse.mybir as mybir
import concourse.tile as tile
from concourse.bass_utils import run_bass_kernel_spmd

N = 208                      # nodes
HALF = 104                   # output nodes per block
K = 4                        # band half-width of S
NH = 2 * K + HALF            # 112 contraction rows per block (halo incl.)
NST = 104                    # store rows per block (no pad)
N_CORES = 8
B, T = 64, 2048
ROWS_TOTAL = B * T           # 131072
SHARD = ROWS_TOTAL // N_CORES    # 16384 rows per core
TB = 512                     # moving-block columns per matmul (fp32 PSUM bank)
TG = 2 * TB                  # psum tile / eviction group (2 banks)
# tapered chunk schedule: big chunks in the middle for DMA efficiency,
# small ones at the end so the post-last-load compute+store chain is
# short (the chain after the final load gates the kernel end)
CHUNKS = [2048, 4096, 4096, 4096, 1024, 512, 512]
assert sum(CHUNKS) == SHARD
N_CHUNKS = len(CHUNKS)
LOAD_PIECE = 2048            # 4KB lines: reads run ~14% faster than 8KB

FP32 = mybir.dt.float32
BF16 = mybir.dt.bfloat16
BF16_NP = mybir.dt.np(BF16)

# halo row order (indices into the [208] node dim) for each block
ROWS0 = list(range(N - K, N)) + list(range(0, HALF + K))          # 112
ROWS1 = list(range(HALF - K, N)) + list(range(0, K))              # 112

_CACHE = {}
LAST_RESULTS = None          # BassKernelResults of the most recent run


def _kernel_body(tc):
    nc = tc.nc
    # rows 0:112 block0 halo, 112:224 block1 halo
    x_d = nc.dram_tensor("xh", [2 * NH, SHARD], BF16, kind="ExternalInput").ap()
    w_d = nc.dram_tensor("wh", [NH, N], BF16, kind="ExternalInput").ap()
    b_d = nc.dram_tensor("bias", [1, N], FP32, kind="ExternalInput").ap()
    o_d = nc.dram_tensor("outt", [2 * NST, SHARD], BF16, kind="ExternalOutput").ap()

    with ExitStack() as ctx:
        const = ctx.enter_context(tc.tile_pool(name="const", bufs=1))

        # Setup rides the Scalar HWDGE ring (idle until the first store)
        # so the Sync ring's first item is the first x chunk.
        wh = const.tile([NH, N], BF16, tag="wh")
        nc.scalar.dma_start(wh, w_d)
        bA = const.tile([HALF, 1], FP32, tag="bA")
        bB = const.tile([HALF, 1], FP32, tag="bB")
        b_col = b_d.rearrange("o n -> n o")
        nc.scalar.dma_start(bA, b_col[0:HALF, :])
        nc.scalar.dma_start(bB, b_col[HALF:N, :])
        wh0 = wh[:, 0:HALF]
        wh1 = wh[:, HALF:N]

        # full-shard prefetch: every load issues before compute starts
        x0p = ctx.enter_context(tc.tile_pool(name="x0p", bufs=N_CHUNKS))
        x1p = ctx.enter_context(tc.tile_pool(name="x1p", bufs=N_CHUNKS))
        o0p = ctx.enter_context(tc.tile_pool(name="o0p", bufs=2))
        o1p = ctx.enter_context(tc.tile_pool(name="o1p", bufs=2))
        psp = ctx.enter_context(tc.tile_pool(name="psp", bufs=4, space="PSUM"))

        offs = [sum(CHUNKS[:c]) for c in range(N_CHUNKS)]
        xh0s, xh1s = [], []
        for c, (lo, tw) in enumerate(zip(offs, CHUNKS)):
            xh0 = x0p.tile([NH, tw], BF16, tag="xh0")
            xh1 = x1p.tile([NH, tw], BF16, tag="xh1")
            # loads in <=4KB-line pieces (reads run ~14% faster than 8KB)
            for p0 in range(0, tw, LOAD_PIECE):
                p1 = min(p0 + LOAD_PIECE, tw)
                nc.sync.dma_start(xh0[:, p0:p1], x_d[0:NH, lo + p0 : lo + p1])
                nc.sync.dma_start(xh1[:, p0:p1], x_d[NH : 2 * NH, lo + p0 : lo + p1])
            xh0s.append(xh0)
            xh1s.append(xh1)

        evict_ctr = 0
        for c, (lo, tw) in enumerate(zip(offs, CHUNKS)):
            tsl = slice(lo, lo + tw)
            o0_t = o0p.tile([NST, tw], BF16, tag="o0")
            o1_t = o1p.tile([NST, tw], BF16, tag="o1")
            for blk, (whe, xt, o_t, be) in enumerate(
                [(wh0, xh0s[c], o0_t, bA), (wh1, xh1s[c], o1_t, bB)]
            ):
                for g0 in range(0, tw, TG):
                    gw = min(TG, tw - g0)
                    ps = psp.tile([HALF, gw], FP32, tag="ps")
                    for m0 in range(0, gw, TB):
                        nc.tensor.matmul(
                            ps[:, m0 : m0 + TB],
                            whe,
                            xt[:, g0 + m0 : g0 + m0 + TB],
                            start=True,
                            stop=True,
                        )
                    osl = o_t[:, g0 : g0 + gw]
                    # alternate evictions between VectorE and ScalarE
                    if evict_ctr % 2 == 0:
                        nc.vector.tensor_scalar_add(osl, ps, be)
                    else:
                        nc.scalar.add(osl, ps, be)
                    evict_ctr += 1
            if c == N_CHUNKS - 1:
                # tail: last stores ride both rings
                nc.scalar.dma_start(o_d[0:NST, tsl], o0_t)
                nc.sync.dma_start(o_d[NST : 2 * NST, tsl], o1_t)
            else:
                nc.scalar.dma_start(o_d[0:NST, tsl], o0_t)
                nc.scalar.dma_start(o_d[NST : 2 * NST, tsl], o1_t)


def _build():
    nc = bacc.Bacc(
        "TRN2",
        target_bir_lowering=False,
        debug=False,
        num_devices=N_CORES,
    )
    with tile.TileContext(nc) as tc:
        _kernel_body(tc)
    nc.compile()
    return nc


def kernel(x, W, b, S):
    global LAST_RESULTS
    nc = _CACHE.get("nc")
    if nc is None:
        nc = _build()
        _CACHE["nc"] = nc

    xf = np.asarray(x, np.float32).reshape(ROWS_TOTAL, N)
    WS = np.asarray(S, np.float32) * np.asarray(W, np.float32)
    wh = np.empty((NH, N), BF16_NP)
    wh[:, 0:HALF] = WS[ROWS0][:, 0:HALF].astype(BF16_NP)
    wh[:, HALF:N] = WS[ROWS1][:, HALF:N].astype(BF16_NP)
    bf = np.ascontiguousarray(np.asarray(b, np.float32).reshape(1, N))

    xt = np.ascontiguousarray(xf.T).astype(BF16_NP)      # [208, 131072] bf16
    in_maps = []
    for i in range(N_CORES):
        sl = slice(i * SHARD, (i + 1) * SHARD)
        xh = np.empty((2 * NH, SHARD), BF16_NP)
        xh[0:NH] = xt[ROWS0, sl]
        xh[NH : 2 * NH] = xt[ROWS1, sl]
        in_maps.append({"xh": xh, "wh": wh, "bias": bf})
    res = run_bass_kernel_spmd(nc, in_maps, core_ids=list(range(N_CORES)))
    LAST_RESULTS = res
    out = np.empty((ROWS_TOTAL, N), np.float32)
    for i, r in enumerate(res.results):
        yt = r["outt"]                                   # [208, SHARD] bf16
        out[i * SHARD : (i + 1) * SHARD, 0:HALF] = yt[0:HALF].T.astype(np.float32)
        out[i * SHARD : (i + 1) * SHARD, HALF:N] = yt[HALF:N].T.astype(np.float32)
    return out.reshape(B, T, N)


# revision 9
# speedup vs baseline: 1.0743x; 1.0743x over previous
"""Locally-connected graph-conv kernel for Trainium2 (Bass/Tile), bf16.

Computes out[b,t,m] = sum_n x[b,t,n] * (S*W)[n,m] + bias[m] for
x [64, 2048, 208], W/S [208, 208], bias [208].

The ring-graph support S is a +-4 band (mod 208), so each half of the
output nodes only needs a 112-row slice of the contraction dim:
  block 0 (m 0..103):   n in {204..207} ++ {0..107}
  block 1 (m 104..207): n in {100..207} ++ {0..3}

Tolerance is 2e-2 and bf16 end-to-end measures 4.5e-3 max rel err, so
the host pre-casts x and the masked weight to bf16 (halves HBM load
traffic), the kernel stores bf16 (halves store traffic), and the host
upcasts on gather. The 16 DMA engines saturate at ~20GB/s each
(~320GB/s/core with all 8 cores streaming; reads 17.9, writes 23.1),
so the ~14MB/core of traffic bounds the kernel at ~44us of DMA window
plus ~7us fixed preamble; everything else must hide under it:
  - setup (wh/bias, 47KB) rides the Scalar HWDGE ring ahead of the
    stores; x loads all issue up-front on the Sync ring (deep pools)
    so the load queue never idles and chunk processing is interleaved
    (block0+block1 per chunk) to start stores early and keep
    reads+writes concurrently in the memory controller;
  - PSUM evictions alternate VectorE/ScalarE per group;
  - one 8-bank PSUM pool (bufs=4) keeps 4 matmul groups in flight;
  - stores are unpadded [104, T] (the 16-engine stripe splits 104
    lines 7/6 -- mildly uneven, but saves 7% of store bytes).

Data-parallel over 8 NeuronCores: each core gets 16384 rows of the
flattened x, host-pre-assembled into a bf16 [224, 16384] tensor (two
112-row halo blocks). The host transposes/upcasts at gather.
"""

import numpy as np
from contextlib import ExitStack

import concourse.bacc as bacc
import concourse.mybir as mybir
import concourse.tile as tile
from concourse.bass_utils import run_bass_kernel_spmd

N = 208                      # nodes
HALF = 104                   # output nodes per block
K = 4                        # band half-width of S
NH = 2 * K + HALF            # 112 contraction rows per block (halo incl.)
NST = 104                    # store rows per block (no pad)
N_CORES = 8
B, T = 64, 2048
ROWS_TOTAL = B * T           # 131072
SHARD = ROWS_TOTAL // N_CORES    # 16384 rows per core
TB = 512                     # moving-block columns per matmul (fp32 PSUM bank)
TG = 2 * TB                  # psum tile / eviction group (2 banks)
TOUT = 4096                  # t-columns per DMA chunk (0.92 MB bf16 loads)
N_CHUNKS = SHARD // TOUT     # 4
SUB = TOUT // TG             # 4 psum groups per chunk per block

FP32 = mybir.dt.float32
BF16 = mybir.dt.bfloat16
BF16_NP = mybir.dt.np(BF16)

# halo row order (indices into the [208] node dim) for each block
ROWS0 = list(range(N - K, N)) + list(range(0, HALF + K))          # 112
ROWS1 = list(range(HALF - K, N)) + list(range(0, K))              # 112

_CACHE = {}
LAST_RESULTS = None          # BassKernelResults of the most recent run


def _kernel_body(tc):
    nc = tc.nc
    # rows 0:112 block0 halo, 112:224 block1 halo
    x_d = nc.dram_tensor("xh", [2 * NH, SHARD], BF16, kind="ExternalInput").ap()
    w_d = nc.dram_tensor("wh", [NH, N], BF16, kind="ExternalInput").ap()
    b_d = nc.dram_tensor("bias", [1, N], FP32, kind="ExternalInput").ap()
    o_d = nc.dram_tensor("outt", [2 * NST, SHARD], BF16, kind="ExternalOutput").ap()

    with ExitStack() as ctx:
        const = ctx.enter_context(tc.tile_pool(name="const", bufs=1))

        # Setup rides the Scalar HWDGE ring (idle until the first store)
        # so the Sync ring's first item is the first x chunk.
        wh = const.tile([NH, N], BF16, tag="wh")
        nc.scalar.dma_start(wh, w_d)
        bA = const.tile([HALF, 1], FP32, tag="bA")
        bB = const.tile([HALF, 1], FP32, tag="bB")
        b_col = b_d.rearrange("o n -> n o")
        nc.scalar.dma_start(bA, b_col[0:HALF, :])
        nc.scalar.dma_start(bB, b_col[HALF:N, :])
        wh0 = wh[:, 0:HALF]
        wh1 = wh[:, HALF:N]

        # full-shard prefetch AND full output residency: every load issues
        # before compute starts; every output tile lives until its store
        # drains, so stores can queue BEHIND the loads on the same rings
        # (FIFO = loads get strict priority; the engines run the read-only
        # phase at full rate, then burst the writes)
        x0p = ctx.enter_context(tc.tile_pool(name="x0p", bufs=N_CHUNKS))
        x1p = ctx.enter_context(tc.tile_pool(name="x1p", bufs=N_CHUNKS))
        o0p = ctx.enter_context(tc.tile_pool(name="o0p", bufs=N_CHUNKS))
        o1p = ctx.enter_context(tc.tile_pool(name="o1p", bufs=N_CHUNKS))
        psp = ctx.enter_context(tc.tile_pool(name="psp", bufs=4, space="PSUM"))

        xh0s, xh1s = [], []
        for c in range(N_CHUNKS):
            tsl = slice(c * TOUT, (c + 1) * TOUT)
            xh0 = x0p.tile([NH, TOUT], BF16, tag="xh0")
            if c == 0:
                # head: first matmul only needs the first columns
                h = TOUT // 2
                nc.sync.dma_start(xh0[:, 0:h], x_d[0:NH, 0:h])
                nc.sync.dma_start(xh0[:, h:TOUT], x_d[0:NH, h:TOUT])
            else:
                nc.sync.dma_start(xh0, x_d[0:NH, tsl])
            xh0s.append(xh0)
        for c in range(N_CHUNKS):
            tsl = slice(c * TOUT, (c + 1) * TOUT)
            xh1 = x1p.tile([NH, TOUT], BF16, tag="xh1")
            nc.scalar.dma_start(xh1, x_d[NH : 2 * NH, tsl])
            xh1s.append(xh1)

        for c in range(N_CHUNKS):
            tsl = slice(c * TOUT, (c + 1) * TOUT)
            o0_t = o0p.tile([NST, TOUT], BF16, tag="o0")
            o1_t = o1p.tile([NST, TOUT], BF16, tag="o1")
            for blk, (whe, xt, o_t, be) in enumerate(
                [(wh0, xh0s[c], o0_t, bA), (wh1, xh1s[c], o1_t, bB)]
            ):
                for s in range(SUB):
                    ga = slice(s * TG, s * TG + TB)
                    gb = slice(s * TG + TB, (s + 1) * TG)
                    ps = psp.tile([HALF, TG], FP32, tag="ps")
                    nc.tensor.matmul(ps[:, 0:TB], whe, xt[:, ga], start=True, stop=True)
                    nc.tensor.matmul(ps[:, TB:TG], whe, xt[:, gb], start=True, stop=True)
                    osl = o_t[:, s * TG : (s + 1) * TG]
                    # alternate evictions between VectorE and ScalarE
                    if (blk + s) % 2 == 0:
                        nc.vector.tensor_scalar_add(osl, ps, be)
                    else:
                        nc.scalar.add(osl, ps, be)
            # stores queue behind this ring's loads: strict load priority
            nc.sync.dma_start(o_d[0:NST, tsl], o0_t)
            nc.scalar.dma_start(o_d[NST : 2 * NST, tsl], o1_t)


def _build():
    nc = bacc.Bacc(
        "TRN2",
        target_bir_lowering=False,
        debug=False,
        num_devices=N_CORES,
    )
    with tile.TileContext(nc) as tc:
        _kernel_body(tc)
    nc.compile()
    return nc


def kernel(x, W, b, S):
    global LAST_RESULTS
    nc = _CACHE.get("nc")
    if nc is None:
        nc = _build()
        _CACHE["nc"] = nc

    xf = np.asarray(x, np.float32).reshape(ROWS_TOTAL, N)
    WS = np.asarray(S, np.float32) * np.asarray(W, np.float32)
    wh = np.empty((NH, N), BF16_NP)
    wh[:, 0:HALF] = WS[ROWS0][:, 0:HALF].astype(BF16_NP)
    wh[:, HALF:N] = WS[ROWS1][:, HALF:N].astype(BF16_NP)
    bf = np.ascontiguousarray(np.asarray(b, np.float32).reshape(1, N))

    xt = np.ascontiguousarray(xf.T).astype(BF16_NP)      # [208, 131072] bf16
    in_maps = []
    for i in range(N_CORES):
        sl = slice(i * SHARD, (i + 1) * SHARD)
        xh = np.empty((2 * NH, SHARD), BF16_NP)
        xh[0:NH] = xt[ROWS0, sl]
        xh[NH : 2 * NH] = xt[ROWS1, sl]
        in_maps.append({"xh": xh, "wh": wh, "bias": bf})
    res = run_bass_kernel_spmd(nc, in_maps, core_ids=list(range(N_CORES)))
    LAST_RESULTS = res
    out = np.empty((ROWS_TOTAL, N), np.float32)
    for i, r in enumerate(res.results):
        yt = r["outt"]                                   # [208, SHARD] bf16
        out[i * SHARD : (i + 1) * SHARD, 0:HALF] = yt[0:HALF].T.astype(np.float32)
        out[i * SHARD : (i + 1) * SHARD, HALF:N] = yt[HALF:N].T.astype(np.float32)
    return out.reshape(B, T, N)
